# revision 38
# baseline (speedup 1.0000x reference)
"""CaptioningRNN (LSTM + tiny spatial attention) Trainium2 kernel.

Contract: kernel(**inputs) takes FULL inputs (numpy), returns FULL output
(N, T, H) float32.  Internally: data-parallel over batch N across 8
NeuronCores (16 sequences per core, zero cross-core traffic).

Per-core algorithm:
  phase 0: P[t] = x_t @ Wx + b precomputed for all t (PE, bf16) -> DRAM
  phase 1: 512 sequential LSTM steps.

Key optimizations over the naive schedule:
  * softmax exp computed via the sigmoid identity e^x = sig(x)/(1-sig(x))
    so the Activation engine never switches tables (Sigmoid/Tanh/Copy all
    live in one table; Exp does not -- each switch costs ~1.4us).
  * attn @ Wattn is algebraically rewritten: attn_n = A_n w_n, so
    attn_n @ Wattn = w_n @ B_n with B_n := A_n^T Wattn ([M=16, 4H] per
    sample, precomputed on the host).  The per-step contraction drops
    from K=512 to K=16 (block-diagonal over samples, 2 matmuls instead
    of 16) and the attention-output reduction disappears entirely.
  * per-step PE program order interleaves the gate matmuls with the
    attention colsum/transpose ops so the PE never waits on the
    (Pool/Act/DVE) softmax chain; psum quarters complete staggered
    (g,f,i,o) so activations and the c/h update pipeline behind the PE.
  * all but the first row block of phase 0 is streamed into the
    recurrence loop (one j-quarter per step) to fill PE idle slots
    instead of running ~200us of serial prologue.
"""

import os
import sys
import numpy as np

sys.path.insert(0, "/opt/trn_rl_repo")

import ml_dtypes

BF16 = ml_dtypes.bfloat16

N, T, D, H, M = 128, 512, 512, 512, 16
NCORES = 8
NL = N // NCORES          # 16 sequences per core
KC = 4                    # 512 = 4 chunks of 128 (contraction dims)
J = 4 * H                 # 2048 gate columns
TB = 8                    # time steps per phase-0 row block
RB = NL * T // 128        # phase-0 row blocks (rows = n*TB + tt)
NH = NL // 2              # samples per block-diagonal half

_CACHE = {}


def build(t_steps=T, has_bias=False):
    from concourse import bacc, mybir
    import concourse.tile as tile

    f32 = mybir.dt.float32
    bf16 = mybir.dt.bfloat16
    mult = mybir.AluOpType.mult
    add = mybir.AluOpType.add
    AF = mybir.ActivationFunctionType
    AX = mybir.AxisListType.X

    rb = NL * t_steps // 128

    nc = bacc.Bacc("TRN2", target_bir_lowering=False, debug=False,
                   num_devices=NCORES)

    # ---- I/O -----------------------------------------------------------
    xs = nc.dram_tensor("xs", [rb, 128, KC, 128], bf16, kind="ExternalInput")
    at_d = nc.dram_tensor("at", [128, KC, NL, M], bf16, kind="ExternalInput")
    wx_d = nc.dram_tensor("wx", [128, KC, J], bf16, kind="ExternalInput")
    wh_d = nc.dram_tensor("wh", [128, KC, J], bf16, kind="ExternalInput")
    b_d2 = nc.dram_tensor("bmat", [128, 2, J], bf16, kind="ExternalInput")
    bd_d = nc.dram_tensor("bdmask", [128, 2, NL], bf16, kind="ExternalInput")
    h0_d = nc.dram_tensor("h0t", [128, KC, NL], bf16, kind="ExternalInput")
    c0_d = nc.dram_tensor("c0", [NL, H], f32, kind="ExternalInput")
    id_d = nc.dram_tensor("ident", [NL, NL], bf16, kind="ExternalInput")
    oc_d = nc.dram_tensor("ones_col", [128, 1], bf16, kind="ExternalInput")
    or_d = nc.dram_tensor("ones_row", [1, 128], bf16, kind="ExternalInput")
    if has_bias:
        bias_d = nc.dram_tensor("bvec", [1, J], f32, kind="ExternalInput")
    p_d = nc.dram_tensor("pbuf", [rb, 128, J], bf16)
    out_d = nc.dram_tensor("out", [NL, t_steps, H], bf16,
                           kind="ExternalOutput")

    inv_sqrt_h = float(1.0 / np.sqrt(H))

    # gate column quarters: i=[0:512] f=[512:1024] o=[1024:1536] g=[1536:2048]
    Q_I, Q_F, Q_O, Q_G = 0, 1, 2, 3
    qs = lambda q: slice(q * 512, (q + 1) * 512)

    from contextlib import ExitStack
    with tile.TileContext(nc) as tc, ExitStack() as stack:
        # ---- persistent constants -------------------------------------
        cpool = stack.enter_context(tc.tile_pool(name="consts", bufs=1))
        wh_s = cpool.tile([128, KC, J], bf16)
        bm_s = cpool.tile([128, 2, J], bf16)
        bd_s = cpool.tile([128, 2, NL], bf16)
        at_s = cpool.tile([128, KC, NL, M], bf16)
        h0_s = cpool.tile([128, KC, NL], bf16)
        id_s = cpool.tile([NL, NL], bf16)
        oc_s = cpool.tile([128, 1], bf16)
        or_s = cpool.tile([1, 128], bf16)
        nc.sync.dma_start(out=wh_s[:, :, :], in_=wh_d.ap()[:, :, :])
        nc.sync.dma_start(out=bm_s[:, :, :], in_=b_d2.ap()[:, :, :])
        nc.sync.dma_start(out=bd_s[:, :, :], in_=bd_d.ap()[:, :, :])
        nc.sync.dma_start(out=at_s[:, :, :, :], in_=at_d.ap()[:, :, :, :])
        nc.sync.dma_start(out=h0_s[:, :, :], in_=h0_d.ap()[:, :, :])
        nc.sync.dma_start(out=id_s[:, :], in_=id_d.ap()[:, :])
        nc.sync.dma_start(out=oc_s[:, :], in_=oc_d.ap()[:, :])
        nc.sync.dma_start(out=or_s[:, :], in_=or_d.ap()[:, :])

        wx_s = cpool.tile([128, KC, J], bf16)
        nc.sync.dma_start(out=wx_s[:, :, :], in_=wx_d.ap()[:, :, :])
        # stream all but block 0 of phase 0 into the recurrence (PE idle)
        stream_p0 = (not has_bias) and rb > 1
        rb_pre = 1 if stream_p0 else rb

        # ---- phase 0 preamble: P = x @ Wx (+ b) -----------------------
        with tc.tile_pool(name="ph0", bufs=1) as p0c, \
             tc.tile_pool(name="ph0x", bufs=3) as p0x, \
             tc.tile_pool(name="ph0o", bufs=3) as p0o, \
             tc.tile_pool(name="ps0", bufs=2, space="PSUM") as ps0:
            if has_bias:
                bf_s = p0c.tile([1, J], f32)
                nc.sync.dma_start(out=bf_s[:, :], in_=bias_d.ap()[:, :])
                bb_s = p0c.tile([1, J], bf16)
                nc.vector.tensor_copy(bb_s[:, :], bf_s[:, :])
                psb = ps0.tile([128, J], f32, tag="psb")
                for jt in range(4):
                    nc.tensor.matmul(psb[:, qs(jt)],
                                     or_s[:, :],
                                     bb_s[:, qs(jt)],
                                     start=True, stop=True)
                brep = p0c.tile([128, J], bf16)
                nc.vector.tensor_copy(brep[:, :], psb[:, :])

            for b_i in range(rb_pre):
                xt = p0x.tile([128, KC, 128], bf16, tag="xt")
                nc.sync.dma_start(out=xt[:, :, :], in_=xs.ap()[b_i, :, :, :])
                psp = ps0.tile([128, J], f32, tag="psp")
                for kc in range(KC):
                    for jt in range(4):
                        nc.tensor.matmul(
                            psp[:, qs(jt)],
                            xt[:, kc, :],
                            wx_s[:, kc, qs(jt)],
                            start=(kc == 0), stop=(kc == KC - 1))
                pout = p0o.tile([128, J], bf16, tag="pout")
                for jt in range(4):
                    sl = qs(jt)
                    if has_bias:
                        nc.vector.tensor_tensor(pout[:, sl], psp[:, sl],
                                                brep[:, sl], add)
                    elif jt in (1, 3):
                        nc.scalar.copy(pout[:, sl], psp[:, sl])
                    else:
                        nc.vector.tensor_copy(pout[:, sl], psp[:, sl])
                nc.sync.dma_start(out=p_d.ap()[b_i, :, :], in_=pout[:, :])

        # ---- phase 1: recurrence --------------------------------------
        with tc.tile_pool(name="state", bufs=2) as stp, \
             tc.tile_pool(name="work", bufs=2) as wk, \
             tc.tile_pool(name="pin", bufs=4) as pin, \
             tc.tile_pool(name="ps_a", bufs=1, space="PSUM") as psa_p, \
             tc.tile_pool(name="ps_s", bufs=1, space="PSUM") as pss:

            c_t = stp.tile([NL, H], f32, tag="c")
            nc.sync.dma_start(out=c_t[:, :], in_=c0_d.ap()[:, :])
            hT = h0_s

            if stream_p0:
                xt_nxt = pin.tile([128, KC, 128], bf16, tag="xts",
                                  name="xt_nxt")
                nc.sync.dma_start(out=xt_nxt[:, :, :],
                                  in_=xs.ap()[1, :, :, :])
                xt_cur = None

            for t in range(t_steps):
                b_i, tt = divmod(t, TB)
                p_t = pin.tile([NL, J], bf16, tag="pt")
                nc.sync.dma_start(out=p_t[:, :],
                                  in_=p_d.ap()[b_i, tt * NL:(tt + 1) * NL, :])

                # -- attention scores: s2 on Pool (kc-halves), colsum on PE
                s2 = wk.tile([128, KC, NL, M], bf16, tag="s2")
                for kh in range(2):
                    ks = slice(2 * kh, 2 * kh + 2)
                    nc.gpsimd.tensor_tensor(
                        s2[:, ks, :, :], at_s[:, ks, :, :],
                        hT[:, ks, :, None].broadcast_to([128, 2, NL, M]),
                        mult)

                # psum quarter tiles (1 bank each, tag-rotated)
                pq = [psa_p.tile([NL, 512], f32, tag=f"q{q}",
                                 name=f"pq{q}")
                      for q in range(4)]

                def gates_q(q):
                    nc.tensor.matmul(pq[q][:, :], id_s[:, :],
                                     p_t[:, qs(q)], start=True, stop=False)
                    for kc in range(KC):
                        nc.tensor.matmul(pq[q][:, :], hT[:, kc, :],
                                         wh_s[:, kc, qs(q)],
                                         start=False, stop=False)

                def attn_q(q):
                    for half in range(2):
                        nc.tensor.matmul(pq[q][:, :], S_t[:, half, :],
                                         bm_s[:, half, qs(q)],
                                         start=False, stop=(half == 1))

                # PE: quarter f gates first (overlaps softmax chain)
                gates_q(Q_F)
                psz = pss.tile([1, NL, M], f32, tag="z")
                for half in range(2):
                    sh = slice(half * NH, (half + 1) * NH)
                    for kc in range(KC):
                        nc.tensor.matmul(psz[:, sh, :], oc_s[:, :],
                                         s2[:, kc, sh, :],
                                         start=(kc == 0), stop=(kc == KC - 1))
                gates_q(Q_I)
                gates_q(Q_G)

                # softmax via sigmoid identity, pipelined by sample halves
                u_t = wk.tile([1, NL, M], bf16, tag="u")
                v_t = wk.tile([1, NL, M], bf16, tag="v")
                e_t = wk.tile([1, NL, M], bf16, tag="e")
                sum_e = wk.tile([1, NL, 1], f32, tag="sume")
                rec = wk.tile([1, NL, 1], f32, tag="rec")
                w_t = wk.tile([1, NL * M], bf16, tag="wt")
                pwt = pss.tile([128, 2, 2], bf16, tag="wT")
                S_t = wk.tile([128, 2, NL], bf16, tag="S")
                nc.scalar.activation(u_t[:, :, :], psz[:, :, :],
                                     AF.Sigmoid, scale=inv_sqrt_h)
                nc.vector.tensor_scalar(v_t[:, :, :], u_t[:, :, :],
                                        -1.0, 1.0, mult, add)
                with nc.allow_low_precision(reason="M=16 softmax weights"):
                    nc.vector.reciprocal(v_t[:, :, :], v_t[:, :, :])
                    nc.vector.tensor_tensor(e_t[:, :, :], u_t[:, :, :],
                                            v_t[:, :, :], mult)
                    nc.vector.tensor_reduce(sum_e[:, :, :], e_t[:, :, :],
                                            AX, add)
                nc.vector.reciprocal(rec[:, :, :], sum_e[:, :, :])
                nc.vector.tensor_tensor(
                    w_t.rearrange("p (nl m) -> p nl m", nl=NL),
                    e_t[:, :, :],
                    rec[:, :, :].broadcast_to([1, NL, M]), mult)
                for half in range(2):
                    nc.tensor.transpose(
                        pwt[:, half, 0:1],
                        w_t[:, half * 128:(half + 1) * 128],
                        oc_s[0:1, 0:1])
                nc.vector.tensor_tensor(
                    S_t[:, :, :],
                    pwt[:, :, 0:1].broadcast_to([128, 2, NL]),
                    bd_s[:, :, :], mult)

                gates_q(Q_O)

                attn_q(Q_G)
                attn_q(Q_F)
                attn_q(Q_I)
                attn_q(Q_O)

                # -- activations + state update, half-H pipelined
                # (quarters land g,f,i,o; act queue: g,f,i then c/o halves)
                sig_f = wk.tile([NL, H], bf16, tag="sf")
                sig_i = wk.tile([NL, H], bf16, tag="si")
                tan_g = wk.tile([NL, H], bf16, tag="tg")
                sig_o = wk.tile([NL, H], bf16, tag="so")
                t1 = wk.tile([NL, H], f32, tag="t1")
                t2 = wk.tile([NL, H], bf16, tag="t2")
                c_n = stp.tile([NL, H], f32, tag="c")
                tan_c = wk.tile([NL, H], bf16, tag="tc")
                h_bf = wk.tile([NL, H], bf16, tag="hbf")
                HH = H // 2
                hs = [slice(0, HH), slice(HH, H)]

                nc.scalar.activation(tan_g[:, :], pq[Q_G][:, :], AF.Tanh)
                nc.scalar.activation(sig_f[:, :], pq[Q_F][:, :], AF.Sigmoid)
                for hh in range(2):
                    sl = hs[hh]
                    nc.gpsimd.tensor_tensor(t1[:, sl], sig_f[:, sl],
                                            c_t[:, sl], mult)
                    nc.scalar.activation(sig_i[:, sl], pq[Q_I][:, sl],
                                         AF.Sigmoid)
                    nc.vector.tensor_tensor(t2[:, sl], sig_i[:, sl],
                                            tan_g[:, sl], mult)
                    nc.vector.tensor_tensor(c_n[:, sl], t1[:, sl],
                                            t2[:, sl], add)
                    nc.scalar.activation(tan_c[:, sl], c_n[:, sl], AF.Tanh)
                    nc.scalar.activation(sig_o[:, sl], pq[Q_O][:, sl],
                                         AF.Sigmoid)
                    nc.vector.tensor_tensor(h_bf[:, sl], sig_o[:, sl],
                                            tan_c[:, sl], mult)
                nc.sync.dma_start(out=out_d.ap()[:, t, :], in_=h_bf[:, :])

                pst = pss.tile([128, KC * NL], bf16, tag="tr")
                hT_n = stp.tile([128, KC, NL], bf16, tag="hT")
                for hh in range(2):
                    for kc in (2 * hh, 2 * hh + 1):
                        nc.tensor.transpose(pst[:, kc * NL:(kc + 1) * NL],
                                            h_bf[:, kc * 128:(kc + 1) * 128],
                                            id_s[:, :])
                    nc.vector.tensor_copy(
                        hT_n[:, 2 * hh:2 * hh + 2, :],
                        pst[:, 2 * hh * NL:(2 * hh + 2) * NL].rearrange(
                            "p (kc nl) -> p kc nl", kc=2))

                # streamed phase-0 chunk: one j-quarter of one row block
                if stream_p0 and t < 4 * (rb - 1):
                    sb, sjt = divmod(t, 4)
                    sb += 1
                    if sjt == 0:
                        xt_cur = xt_nxt
                        if sb + 1 < rb:
                            xt_nxt = pin.tile([128, KC, 128], bf16,
                                              tag="xts", name="xt_nxt")
                            nc.sync.dma_start(out=xt_nxt[:, :, :],
                                              in_=xs.ap()[sb + 1, :, :, :])
                    psq = pss.tile([128, 512], f32, tag="p0q", name="psq")
                    for kc in range(KC):
                        nc.tensor.matmul(psq[:, :], xt_cur[:, kc, :],
                                         wx_s[:, kc, qs(sjt)],
                                         start=(kc == 0), stop=(kc == KC - 1))
                    pco = wk.tile([128, 512], bf16, tag="p0o", name="pco")
                    nc.vector.tensor_copy(pco[:, :], psq[:, :])
                    nc.sync.dma_start(out=p_d.ap()[sb, :, qs(sjt)],
                                      in_=pco[:, :])

                hT = hT_n
                c_t = c_n

    nc.compile()
    return nc


def _stage_inputs(x, A, Wx, Wh, Wattn, b, t_steps=T):
    """Shard + lay out inputs per core (host-side numpy staging)."""
    rb = NL * t_steps // 128
    h0 = A.mean(axis=(2, 3)).astype(np.float32)          # (N, H)
    ident = np.eye(NL, dtype=BF16)
    ones_col = np.ones((128, 1), dtype=BF16)
    ones_row = np.ones((1, 128), dtype=BF16)

    def wlay(w):
        return np.ascontiguousarray(
            w.astype(BF16).reshape(KC, 128, J).transpose(1, 0, 2))

    wxs, whs = wlay(Wx), wlay(Wh)
    bvec = np.ascontiguousarray(b.astype(np.float32).reshape(1, J))

    # block-diagonal mask for S: bd[p, half, col] = 1 iff col == half*NH + p//M
    pp = np.arange(128) // M
    bd = np.zeros((128, 2, NL), dtype=BF16)
    for half in range(2):
        bd[np.arange(128), half, half * NH + pp] = 1

    Wab = Wattn.astype(np.float32)

    maps = []
    for k in range(NCORES):
        ns = slice(k * NL, (k + 1) * NL)
        x_sh = x[ns, :t_steps].astype(BF16)              # (NL, t, D)
        # (tb, p, kc, n*TB+tt)
        xT = x_sh.transpose(2, 0, 1).reshape(KC, 128, NL, rb, TB)
        # row order within a block: r = tt*NL + n
        xs_st = np.ascontiguousarray(
            xT.transpose(3, 1, 0, 4, 2).reshape(rb, 128, KC, 128))
        A_sh = A[ns].reshape(NL, H, M).astype(BF16)
        at_st = np.ascontiguousarray(
            A_sh.transpose(1, 0, 2).reshape(KC, 128, NL, M)
            .transpose(1, 0, 2, 3))
        h0_sh = h0[ns]                                    # (NL, H)
        h0t = np.ascontiguousarray(
            h0_sh.T.astype(BF16).reshape(KC, 128, NL).transpose(1, 0, 2))
        # B_n = A_n^T @ Wattn  -> [NL, M, J]; half h rows = samples h*NH..
        Bn = np.einsum("nhm,hj->nmj", A[ns].reshape(NL, H, M).astype(np.float32),
                       Wab, optimize=True)               # (NL, M, J)
        bmat = np.ascontiguousarray(
            Bn.reshape(2, NH * M, J).transpose(1, 0, 2).astype(BF16))
        m = {
            "xs": xs_st, "at": at_st, "wx": wxs, "wh": whs,
            "bmat": bmat, "bdmask": bd,
            "h0t": h0t, "c0": np.ascontiguousarray(h0_sh),
            "ident": ident, "ones_col": ones_col, "ones_row": ones_row,
        }
        if np.any(b != 0):
            m["bvec"] = bvec
        maps.append(m)
    return maps


def _get_nc(has_bias, t_steps=T):
    key = (has_bias, t_steps)
    if key not in _CACHE:
        _CACHE[key] = build(t_steps=t_steps, has_bias=has_bias)
    return _CACHE[key]


def run_cores(x, A, Wx, Wh, Wattn, b, t_steps=T, trace=False):
    from concourse.bass_utils import run_bass_kernel_spmd
    maps = _stage_inputs(x, A, Wx, Wh, Wattn, b, t_steps=t_steps)
    has_bias = "bvec" in maps[0]
    nc = _get_nc(has_bias, t_steps)
    res = run_bass_kernel_spmd(nc, maps, list(range(NCORES)), trace=trace)
    out = np.concatenate([res.results[k]["out"] for k in range(NCORES)],
                         axis=0)
    return np.asarray(out, dtype=np.float32), res


def kernel(x, A, Wx, Wh, Wattn, b):
    x = np.asarray(x, dtype=np.float32)
    A = np.asarray(A, dtype=np.float32)
    out, _ = run_cores(x, A,
                       np.asarray(Wx, dtype=np.float32),
                       np.asarray(Wh, dtype=np.float32),
                       np.asarray(Wattn, dtype=np.float32),
                       np.asarray(b, dtype=np.float32))
    return out


# revision 44
# speedup vs baseline: 1.4171x; 1.4171x over previous
"""CaptioningRNN (LSTM + tiny spatial attention) Trainium2 kernel.

Contract: kernel(**inputs) takes FULL inputs (numpy), returns FULL output
(N, T, H) float32.  Internally: data-parallel over batch N across 8
NeuronCores (16 sequences per core, zero cross-core traffic).

Per-core algorithm:
  phase 0: P[t] = x_t @ Wx + b precomputed for all t (PE, bf16) -> DRAM
  phase 1: 512 sequential LSTM steps.

Key optimizations over the naive schedule:
  * softmax exp computed via the sigmoid identity e^x = sig(x)/(1-sig(x))
    so the Activation engine never switches tables (Sigmoid/Tanh/Copy all
    live in one table; Exp does not -- each switch costs ~1.4us).
  * attn @ Wattn is algebraically rewritten: attn_n = A_n w_n, so
    attn_n @ Wattn = w_n @ B_n with B_n := A_n^T Wattn ([M=16, 4H] per
    sample, precomputed on the host).  The per-step contraction drops
    from K=512 to K=16 (block-diagonal over samples, 2 matmuls instead
    of 16) and the attention-output reduction disappears entirely.
  * per-step PE program order interleaves the gate matmuls with the
    attention colsum/transpose ops so the PE never waits on the
    (Pool/Act/DVE) softmax chain; psum quarters complete staggered
    (g,f,i,o) so activations and the c/h update pipeline behind the PE.
  * all but the first row block of phase 0 is streamed into the
    recurrence loop (one j-quarter per step) to fill PE idle slots
    instead of running ~200us of serial prologue.
"""

import os
import sys
import numpy as np

sys.path.insert(0, "/opt/trn_rl_repo")

import ml_dtypes

BF16 = ml_dtypes.bfloat16

N, T, D, H, M = 128, 512, 512, 512, 16
NCORES = 8
NL = N // NCORES          # 16 sequences per core
KC = 4                    # 512 = 4 chunks of 128 (contraction dims)
J = 4 * H                 # 2048 gate columns
TB = 8                    # time steps per phase-0 row block
RB = NL * T // 128        # phase-0 row blocks (rows = n*TB + tt)
NH = NL // 2              # samples per block-diagonal half

_CACHE = {}


def build(t_steps=T, has_bias=False):
    from concourse import bacc, mybir
    import concourse.tile as tile

    f32 = mybir.dt.float32
    bf16 = mybir.dt.bfloat16
    mult = mybir.AluOpType.mult
    add = mybir.AluOpType.add
    AF = mybir.ActivationFunctionType
    AX = mybir.AxisListType.X

    rb = NL * t_steps // 128

    nc = bacc.Bacc("TRN2", target_bir_lowering=False, debug=False,
                   num_devices=NCORES)

    # ---- I/O -----------------------------------------------------------
    xs = nc.dram_tensor("xs", [rb, 128, KC, 128], bf16, kind="ExternalInput")
    at_d = nc.dram_tensor("at", [128, KC, NL, M], bf16, kind="ExternalInput")
    wx_d = nc.dram_tensor("wx", [128, KC, J], bf16, kind="ExternalInput")
    wh_d = nc.dram_tensor("wh", [128, KC, J], bf16, kind="ExternalInput")
    b_d2 = nc.dram_tensor("bmat", [128, 2, J], bf16, kind="ExternalInput")
    bd_d = nc.dram_tensor("bdmask", [128, 2, NL], bf16, kind="ExternalInput")
    h0_d = nc.dram_tensor("h0t", [128, KC, NL], bf16, kind="ExternalInput")
    c0_d = nc.dram_tensor("c0", [NL, H], f32, kind="ExternalInput")
    id_d = nc.dram_tensor("ident", [NL, NL], bf16, kind="ExternalInput")
    oc_d = nc.dram_tensor("ones_col", [128, 1], bf16, kind="ExternalInput")
    or_d = nc.dram_tensor("ones_row", [1, 128], bf16, kind="ExternalInput")
    if has_bias:
        bias_d = nc.dram_tensor("bvec", [1, J], f32, kind="ExternalInput")
    p_d = nc.dram_tensor("pbuf", [rb, 128, J], bf16)
    out_d = nc.dram_tensor("out", [NL, t_steps, H], bf16,
                           kind="ExternalOutput")

    inv_sqrt_h = float(1.0 / np.sqrt(H))

    # gate column quarters: i=[0:512] f=[512:1024] o=[1024:1536] g=[1536:2048]
    Q_I, Q_F, Q_O, Q_G = 0, 1, 2, 3
    qs = lambda q: slice(q * 512, (q + 1) * 512)

    from contextlib import ExitStack
    with tile.TileContext(nc) as tc, ExitStack() as stack:
        # ---- persistent constants -------------------------------------
        cpool = stack.enter_context(tc.tile_pool(name="consts", bufs=1))
        wh_s = cpool.tile([128, KC, J], bf16)
        bm_s = cpool.tile([128, 2, J], bf16)
        bd_s = cpool.tile([128, 2, NL], bf16)
        at_s = cpool.tile([128, KC, NL, M], bf16)
        h0_s = cpool.tile([128, KC, NL], bf16)
        id_s = cpool.tile([NL, NL], bf16)
        oc_s = cpool.tile([128, 1], bf16)
        or_s = cpool.tile([1, 128], bf16)
        # constant loads issued from four different engine sequencers so
        # the ~1.6us-per-issue DMA dispatch cost parallelizes at startup
        nc.sync.dma_start(out=wh_s[:, :, :], in_=wh_d.ap()[:, :, :])
        nc.scalar.dma_start(out=bm_s[:, :, :], in_=b_d2.ap()[:, :, :])
        nc.gpsimd.dma_start(out=bd_s[:, :, :], in_=bd_d.ap()[:, :, :])
        nc.gpsimd.dma_start(out=at_s[:, :, :, :], in_=at_d.ap()[:, :, :, :])
        nc.gpsimd.dma_start(out=h0_s[:, :, :], in_=h0_d.ap()[:, :, :])
        nc.scalar.dma_start(out=id_s[:, :], in_=id_d.ap()[:, :])
        nc.scalar.dma_start(out=oc_s[:, :], in_=oc_d.ap()[:, :])
        nc.scalar.dma_start(out=or_s[:, :], in_=or_d.ap()[:, :])

        wx_s = cpool.tile([128, KC, J], bf16)
        nc.gpsimd.dma_start(out=wx_s[:, :, :], in_=wx_d.ap()[:, :, :])
        # stream all but block 0 of phase 0 into the recurrence (PE idle)
        stream_p0 = (not has_bias) and rb > 1
        rb_pre = 1 if stream_p0 else rb

        # ---- phase 0 preamble: P = x @ Wx (+ b) -----------------------
        with tc.tile_pool(name="ph0", bufs=1) as p0c, \
             tc.tile_pool(name="ph0x", bufs=3) as p0x, \
             tc.tile_pool(name="ph0o", bufs=3) as p0o, \
             tc.tile_pool(name="ps0", bufs=2, space="PSUM") as ps0:
            if has_bias:
                bf_s = p0c.tile([1, J], f32)
                nc.sync.dma_start(out=bf_s[:, :], in_=bias_d.ap()[:, :])
                bb_s = p0c.tile([1, J], bf16)
                nc.vector.tensor_copy(bb_s[:, :], bf_s[:, :])
                psb = ps0.tile([128, J], f32, tag="psb")
                for jt in range(4):
                    nc.tensor.matmul(psb[:, qs(jt)],
                                     or_s[:, :],
                                     bb_s[:, qs(jt)],
                                     start=True, stop=True)
                brep = p0c.tile([128, J], bf16)
                nc.vector.tensor_copy(brep[:, :], psb[:, :])

            for b_i in range(rb_pre):
                xt = p0x.tile([128, KC, 128], bf16, tag="xt")
                nc.sync.dma_start(out=xt[:, :, :], in_=xs.ap()[b_i, :, :, :])
                psp = ps0.tile([128, J], f32, tag="psp")
                for kc in range(KC):
                    for jt in range(4):
                        nc.tensor.matmul(
                            psp[:, qs(jt)],
                            xt[:, kc, :],
                            wx_s[:, kc, qs(jt)],
                            start=(kc == 0), stop=(kc == KC - 1))
                pout = p0o.tile([128, J], bf16, tag="pout")
                for jt in range(4):
                    sl = qs(jt)
                    if has_bias:
                        nc.vector.tensor_tensor(pout[:, sl], psp[:, sl],
                                                brep[:, sl], add)
                    elif jt in (1, 3):
                        nc.scalar.copy(pout[:, sl], psp[:, sl])
                    else:
                        nc.vector.tensor_copy(pout[:, sl], psp[:, sl])
                nc.sync.dma_start(out=p_d.ap()[b_i, :, :], in_=pout[:, :])

        # ---- phase 1: recurrence --------------------------------------
        with tc.tile_pool(name="state", bufs=2) as stp, \
             tc.tile_pool(name="work", bufs=2) as wk, \
             tc.tile_pool(name="pin", bufs=4) as pin, \
             tc.tile_pool(name="ps_a", bufs=1, space="PSUM") as psa_p, \
             tc.tile_pool(name="ps_s", bufs=1, space="PSUM") as pss:

            c_t = stp.tile([NL, H], f32, tag="c")
            nc.sync.dma_start(out=c_t[:, :], in_=c0_d.ap()[:, :])
            hT = h0_s

            if stream_p0:
                xt_nxt = pin.tile([128, KC, 128], bf16, tag="xts",
                                  name="xt_nxt")
                nc.sync.dma_start(out=xt_nxt[:, :, :],
                                  in_=xs.ap()[1, :, :, :])
                xt_cur = None

            for t in range(t_steps):
                b_i, tt = divmod(t, TB)
                p_t = pin.tile([NL, J], bf16, tag="pt")
                nc.sync.dma_start(out=p_t[:, :],
                                  in_=p_d.ap()[b_i, tt * NL:(tt + 1) * NL, :])

                # -- attention scores: s2 on Pool (kc-halves), colsum on PE
                s2 = wk.tile([128, KC, NL, M], bf16, tag="s2")
                for kh in range(2):
                    ks = slice(2 * kh, 2 * kh + 2)
                    nc.gpsimd.tensor_tensor(
                        s2[:, ks, :, :], at_s[:, ks, :, :],
                        hT[:, ks, :, None].broadcast_to([128, 2, NL, M]),
                        mult)

                # psum quarter tiles (1 bank each, tag-rotated)
                pq = [psa_p.tile([NL, 512], f32, tag=f"q{q}",
                                 name=f"pq{q}")
                      for q in range(4)]

                def gates_q(q):
                    nc.tensor.matmul(pq[q][:, :], id_s[:, :],
                                     p_t[:, qs(q)], start=True, stop=False)
                    for kc in range(KC):
                        nc.tensor.matmul(pq[q][:, :], hT[:, kc, :],
                                         wh_s[:, kc, qs(q)],
                                         start=False, stop=False)

                def attn_q(q):
                    for half in range(2):
                        nc.tensor.matmul(pq[q][:, :], S_t[:, half, :],
                                         bm_s[:, half, qs(q)],
                                         start=False, stop=(half == 1))

                # PE: quarter f gates first (overlaps softmax chain)
                gates_q(Q_F)
                psz = pss.tile([1, NL, M], f32, tag="z")
                for half in range(2):
                    sh = slice(half * NH, (half + 1) * NH)
                    for kc in range(KC):
                        nc.tensor.matmul(psz[:, sh, :], oc_s[:, :],
                                         s2[:, kc, sh, :],
                                         start=(kc == 0), stop=(kc == KC - 1))
                gates_q(Q_I)
                gates_q(Q_G)

                # softmax via sigmoid identity, pipelined by sample halves
                u_t = wk.tile([1, NL, M], bf16, tag="u")
                v_t = wk.tile([1, NL, M], bf16, tag="v")
                e_t = wk.tile([1, NL, M], bf16, tag="e")
                sum_e = wk.tile([1, NL, 1], f32, tag="sume")
                rec = wk.tile([1, NL, 1], f32, tag="rec")
                w_t = wk.tile([1, NL * M], bf16, tag="wt")
                pwt = pss.tile([128, 2, 2], bf16, tag="wT")
                S_t = wk.tile([128, 2, NL], bf16, tag="S")
                nc.scalar.activation(u_t[:, :, :], psz[:, :, :],
                                     AF.Sigmoid, scale=inv_sqrt_h)
                nc.vector.tensor_scalar(v_t[:, :, :], u_t[:, :, :],
                                        -1.0, 1.0, mult, add)
                with nc.allow_low_precision(reason="M=16 softmax weights"):
                    nc.vector.reciprocal(v_t[:, :, :], v_t[:, :, :])
                    nc.vector.tensor_tensor(e_t[:, :, :], u_t[:, :, :],
                                            v_t[:, :, :], mult)
                    nc.vector.tensor_reduce(sum_e[:, :, :], e_t[:, :, :],
                                            AX, add)
                nc.vector.reciprocal(rec[:, :, :], sum_e[:, :, :])
                nc.vector.tensor_tensor(
                    w_t.rearrange("p (nl m) -> p nl m", nl=NL),
                    e_t[:, :, :],
                    rec[:, :, :].broadcast_to([1, NL, M]), mult)
                for half in range(2):
                    nc.tensor.transpose(
                        pwt[:, half, 0:1],
                        w_t[:, half * 128:(half + 1) * 128],
                        oc_s[0:1, 0:1])
                nc.vector.tensor_tensor(
                    S_t[:, :, :],
                    pwt[:, :, 0:1].broadcast_to([128, 2, NL]),
                    bd_s[:, :, :], mult)

                gates_q(Q_O)

                attn_q(Q_G)
                attn_q(Q_F)
                attn_q(Q_I)
                attn_q(Q_O)

                # -- activations + state update, half-H pipelined
                # (quarters land g,f,i,o; act queue: g,f,i then c/o halves)
                sig_f = wk.tile([NL, H], bf16, tag="sf")
                sig_i = wk.tile([NL, H], bf16, tag="si")
                tan_g = wk.tile([NL, H], bf16, tag="tg")
                sig_o = wk.tile([NL, H], bf16, tag="so")
                t1 = wk.tile([NL, H], f32, tag="t1")
                t2 = wk.tile([NL, H], bf16, tag="t2")
                c_n = stp.tile([NL, H], f32, tag="c")
                tan_c = wk.tile([NL, H], bf16, tag="tc")
                h_bf = wk.tile([NL, H], bf16, tag="hbf")
                HH = H // 2
                hs = [slice(0, HH), slice(HH, H)]

                nc.scalar.activation(tan_g[:, :], pq[Q_G][:, :], AF.Tanh)
                nc.scalar.activation(sig_f[:, :], pq[Q_F][:, :], AF.Sigmoid)
                for hh in range(2):
                    sl = hs[hh]
                    nc.gpsimd.tensor_tensor(t1[:, sl], sig_f[:, sl],
                                            c_t[:, sl], mult)
                    nc.scalar.activation(sig_i[:, sl], pq[Q_I][:, sl],
                                         AF.Sigmoid)
                    nc.vector.tensor_tensor(t2[:, sl], sig_i[:, sl],
                                            tan_g[:, sl], mult)
                    nc.vector.tensor_tensor(c_n[:, sl], t1[:, sl],
                                            t2[:, sl], add)
                    nc.scalar.activation(tan_c[:, sl], c_n[:, sl], AF.Tanh)
                    nc.scalar.activation(sig_o[:, sl], pq[Q_O][:, sl],
                                         AF.Sigmoid)
                    nc.vector.tensor_tensor(h_bf[:, sl], sig_o[:, sl],
                                            tan_c[:, sl], mult)
                nc.sync.dma_start(out=out_d.ap()[:, t, :], in_=h_bf[:, :])

                pst = pss.tile([128, KC * NL], bf16, tag="tr")
                hT_n = stp.tile([128, KC, NL], bf16, tag="hT")
                for hh in range(2):
                    for kc in (2 * hh, 2 * hh + 1):
                        nc.tensor.transpose(pst[:, kc * NL:(kc + 1) * NL],
                                            h_bf[:, kc * 128:(kc + 1) * 128],
                                            id_s[:, :])
                    nc.vector.tensor_copy(
                        hT_n[:, 2 * hh:2 * hh + 2, :],
                        pst[:, 2 * hh * NL:(2 * hh + 2) * NL].rearrange(
                            "p (kc nl) -> p kc nl", kc=2))

                # streamed phase-0 chunk: one j-quarter of one row block
                if stream_p0 and t < 4 * (rb - 1):
                    sb, sjt = divmod(t, 4)
                    sb += 1
                    if sjt == 0:
                        xt_cur = xt_nxt
                        if sb + 1 < rb:
                            xt_nxt = pin.tile([128, KC, 128], bf16,
                                              tag="xts", name="xt_nxt")
                            nc.sync.dma_start(out=xt_nxt[:, :, :],
                                              in_=xs.ap()[sb + 1, :, :, :])
                    psq = pss.tile([128, 512], f32, tag="p0q", name="psq")
                    for kc in range(KC):
                        nc.tensor.matmul(psq[:, :], xt_cur[:, kc, :],
                                         wx_s[:, kc, qs(sjt)],
                                         start=(kc == 0), stop=(kc == KC - 1))
                    pco = wk.tile([128, 512], bf16, tag="p0o", name="pco")
                    nc.vector.tensor_copy(pco[:, :], psq[:, :])
                    nc.sync.dma_start(out=p_d.ap()[sb, :, qs(sjt)],
                                      in_=pco[:, :])

                hT = hT_n
                c_t = c_n

    nc.compile()
    return nc


def _stage_inputs(x, A, Wx, Wh, Wattn, b, t_steps=T):
    """Shard + lay out inputs per core (host-side numpy staging)."""
    rb = NL * t_steps // 128
    h0 = A.mean(axis=(2, 3)).astype(np.float32)          # (N, H)
    ident = np.eye(NL, dtype=BF16)
    ones_col = np.ones((128, 1), dtype=BF16)
    ones_row = np.ones((1, 128), dtype=BF16)

    def wlay(w):
        return np.ascontiguousarray(
            w.astype(BF16).reshape(KC, 128, J).transpose(1, 0, 2))

    wxs, whs = wlay(Wx), wlay(Wh)
    bvec = np.ascontiguousarray(b.astype(np.float32).reshape(1, J))

    # block-diagonal mask for S: bd[p, half, col] = 1 iff col == half*NH + p//M
    pp = np.arange(128) // M
    bd = np.zeros((128, 2, NL), dtype=BF16)
    for half in range(2):
        bd[np.arange(128), half, half * NH + pp] = 1

    Wab = Wattn.astype(np.float32)

    maps = []
    for k in range(NCORES):
        ns = slice(k * NL, (k + 1) * NL)
        x_sh = x[ns, :t_steps].astype(BF16)              # (NL, t, D)
        # (tb, p, kc, n*TB+tt)
        xT = x_sh.transpose(2, 0, 1).reshape(KC, 128, NL, rb, TB)
        # row order within a block: r = tt*NL + n
        xs_st = np.ascontiguousarray(
            xT.transpose(3, 1, 0, 4, 2).reshape(rb, 128, KC, 128))
        A_sh = A[ns].reshape(NL, H, M).astype(BF16)
        at_st = np.ascontiguousarray(
            A_sh.transpose(1, 0, 2).reshape(KC, 128, NL, M)
            .transpose(1, 0, 2, 3))
        h0_sh = h0[ns]                                    # (NL, H)
        h0t = np.ascontiguousarray(
            h0_sh.T.astype(BF16).reshape(KC, 128, NL).transpose(1, 0, 2))
        # B_n = A_n^T @ Wattn  -> [NL, M, J]; half h rows = samples h*NH..
        Bn = np.einsum("nhm,hj->nmj", A[ns].reshape(NL, H, M).astype(np.float32),
                       Wab, optimize=True)               # (NL, M, J)
        bmat = np.ascontiguousarray(
            Bn.reshape(2, NH * M, J).transpose(1, 0, 2).astype(BF16))
        m = {
            "xs": xs_st, "at": at_st, "wx": wxs, "wh": whs,
            "bmat": bmat, "bdmask": bd,
            "h0t": h0t, "c0": np.ascontiguousarray(h0_sh),
            "ident": ident, "ones_col": ones_col, "ones_row": ones_row,
        }
        if np.any(b != 0):
            m["bvec"] = bvec
        maps.append(m)
    return maps


def _get_nc(has_bias, t_steps=T):
    key = (has_bias, t_steps)
    if key not in _CACHE:
        _CACHE[key] = build(t_steps=t_steps, has_bias=has_bias)
    return _CACHE[key]


def run_cores(x, A, Wx, Wh, Wattn, b, t_steps=T, trace=False):
    from concourse.bass_utils import run_bass_kernel_spmd
    maps = _stage_inputs(x, A, Wx, Wh, Wattn, b, t_steps=t_steps)
    has_bias = "bvec" in maps[0]
    nc = _get_nc(has_bias, t_steps)
    res = run_bass_kernel_spmd(nc, maps, list(range(NCORES)), trace=trace)
    out = np.concatenate([res.results[k]["out"] for k in range(NCORES)],
                         axis=0)
    return np.asarray(out, dtype=np.float32), res


def kernel(x, A, Wx, Wh, Wattn, b):
    x = np.asarray(x, dtype=np.float32)
    A = np.asarray(A, dtype=np.float32)
    out, _ = run_cores(x, A,
                       np.asarray(Wx, dtype=np.float32),
                       np.asarray(Wh, dtype=np.float32),
                       np.asarray(Wattn, dtype=np.float32),
                       np.asarray(b, dtype=np.float32))
    return out
